# revision 1
# baseline (speedup 1.0000x reference)
"""Trainium2 Bass kernel for nn_DiffusionBlock (anisotropic diffusion step).

Math (per batch, channel image; s = tau*hx^2, hx = grad kernel tap):
  X[i,j] = u[i,j+1]-u[i,j] (0 at j=W-1),  Y[i,j] = u[i+1,j]-u[i,j] (0 at i=H-1)
  XP/YP  = edge-pad(X/Y) on the (H+2, W+2) grid
  F = a*XP + b*YP,  G = b*XP + c*YP              (padded grid)
  out[i,j] = u[i,j] + s*(F[i+1,j+1]-F[i+1,j] + G[i+1,j+1]-G[i,j+1])

Per-core layout (pure batch data-parallel across 8 cores, 1 batch each):
row-tiles of R=126 output rows. SBUF partition q holds:
  U[q]     = u row r0-1+q (edge-clamped)         [R+2, W]
  A/B/C[q] = a/b/c row r0+q                      [R+1, W+1]
  XT[q]    = X row r0-1+q (DVE free-dim diff)    [R+1, W]
  YT[q]    = Y row r0-1+q (PE bidiagonal matmul) [R+1, W]  (PSUM)
Products (DVE, all partition-aligned, PSUM read for YT):
  PA = A*XTc, PB1 = B*YTc, PB2 = B*XTc, PC = C*YTc   (c = col-clamped shift)
PE assembles the output in PSUM with constant weight matrices (partition
shifts, signs and the scale s all folded into the weights; walrus uses its
fast-FP32 matmul path):
  OUT[p] = U[p+1] + s*((PA+PB1)[p+1]@j+1 - (PA+PB1)[p+1]@j) + Wg@(PB2+PC)
ACT copies PSUM->SBUF, DMA stores.  Boundary clamps are folded into the
DMA row loads (top replicate) and per-tile weight variants of My.
"""

import numpy as np

# Problem geometry (hardcoded per harness contract).
N_CORES = 8
N_CH = 2
H = 1024
W = 1024
R = 126       # output rows per tile
CHUNK = 512   # matmul free-dim chunk (= one PSUM bank of fp32)

_W_NAMES = ("wu", "wsp", "wsn", "wg", "my", "myf", "myl", "myfl")


def _host_weights(s: float, rt_last: int):
    """Constant PE weight matrices, packed [128, 8*128] fp32.

    matmul(out, lhsT, rhs): out[p, n] = sum_k lhsT[k, p] * rhs[k, n]
    """
    k = np.arange(128)[:, None]
    p = np.arange(128)[None, :]
    sf = np.float32(s)
    wu = (k == p + 1).astype(np.float32)            # out[p] += U[p+1]
    wsp = sf * (k == p + 1)                         # out[p] += s * x[p+1]
    wsn = -sf * (k == p + 1)                        # out[p] -= s * x[p+1]
    wg = sf * (k == p + 1) - sf * (k == p)
    my = ((k == p + 1).astype(np.float32) - (k == p))  # YT[q] = U[q+1]-U[q]
    myf = my.copy()                                 # first tile: YT[0] = U[2]-U[1]
    myf[:, 0] = 0.0
    myf[2, 0] = 1.0
    myf[1, 0] = -1.0
    myl = my.copy()                                 # last tile: YT[rt] = 0
    myl[:, rt_last] = 0.0
    myfl = myf.copy()
    myfl[:, rt_last] = 0.0
    mats = {"wu": wu, "wsp": wsp, "wsn": wsn, "wg": wg,
            "my": my, "myf": myf, "myl": myl, "myfl": myfl}
    return np.ascontiguousarray(
        np.concatenate([mats[n].astype(np.float32) for n in _W_NAMES], axis=1)
    )


def _build_nc(n_ch: int, h: int, w: int, r: int, chunk: int, reps: int = 1, mode: str = "full"):
    import concourse.bacc as bacc
    import concourse.mybir as mybir
    import concourse.tile as tile

    f32 = mybir.dt.float32

    nc = bacc.Bacc()
    u_d = nc.dram_tensor("u", [n_ch, h, w], f32, kind="ExternalInput")
    a_d = nc.dram_tensor("a", [n_ch, h + 2, w + 2], f32, kind="ExternalInput")
    b_d = nc.dram_tensor("b", [n_ch, h + 2, w + 2], f32, kind="ExternalInput")
    c_d = nc.dram_tensor("c", [n_ch, h + 2, w + 2], f32, kind="ExternalInput")
    wts_d = nc.dram_tensor("wts", [128, len(_W_NAMES) * 128], f32, kind="ExternalInput")
    out_d = nc.dram_tensor("out", [n_ch, h, w], f32, kind="ExternalOutput")

    tiles = [(r0, min(r, h - r0)) for r0 in range(0, h, r)]

    with tile.TileContext(nc) as tc:
        with (
            tc.tile_pool(name="wpool", bufs=1) as wpool,
            tc.tile_pool(name="io", bufs=3) as io,
            tc.tile_pool(name="tmp", bufs=2) as tmp,
            tc.tile_pool(name="psum", bufs=2, space="PSUM") as psum,
        ):
            # one DMA for all weights, then a barrier so no later instruction
            # ever waits on this DMA (matmul sync-wait slots are scarce)
            w_all = wpool.tile([128, len(_W_NAMES) * 128], f32, tag="w_all")
            nc.sync.dma_start(w_all[:], wts_d[:])
            wt = {
                n: w_all[:, i * 128 : (i + 1) * 128]
                for i, n in enumerate(_W_NAMES)
            }
            # tiny warmup matmul: PE observes the weights DMA here, so no
            # per-tile matmul ever carries that wait (S3_LW wait slots <= 2)
            warm = psum.tile([1, 4], f32, tag="YT")
            with tc.high_priority():
                nc.tensor.matmul(warm[0:1, 0:1], w_all[0:1, 0:1], w_all[0:1, 0:1])

            for _rep in range(reps):
              for ch in range(n_ch):
                for r0, rt in tiles:
                    first = r0 == 0
                    last = r0 + rt == h
                    ka = rt + 1      # A/B/C/XT/YT/product partitions
                    ku = rt + 1 if last else rt + 2  # loaded U partitions
                    # ---- loads ----
                    U = io.tile([128, w], f32, tag="U")
                    lo = r0 - 1
                    clo = max(lo, 0)
                    nc.sync.dma_start(
                        U[clo - lo : ku, :], u_d[ch, clo : lo + ku, :]
                    )
                    if first:
                        nc.sync.dma_start(U[0:1, :], u_d[ch, 0:1, :])
                    # full (w+2)-wide rows: contiguous DRAM block, 1 descriptor
                    A = io.tile([128, w + 2], f32, tag="A")
                    Bt = io.tile([128, w + 2], f32, tag="B")
                    C = io.tile([128, w + 2], f32, tag="C")
                    nc.sync.dma_start(A[0:ka, :], a_d[ch, r0 : r0 + ka, :])
                    nc.sync.dma_start(Bt[0:ka, :], b_d[ch, r0 : r0 + ka, :])
                    nc.sync.dma_start(C[0:ka, :], c_d[ch, r0 : r0 + ka, :])

                    do_xt = mode in ("full", "nope", "nodve", "nomm")
                    do_yt = mode in ("full", "nope", "nodve")
                    do_dve = mode in ("full", "nope", "nomm")
                    do_pe = mode in ("full", "nodve")
                    do_act = mode != "dma"
                    # ---- XT (DVE): free-dim forward diff, col W-1 = 0 ----
                    XT = tmp.tile([128, w], f32, tag="XT")
                    if do_xt:
                        nc.vector.tensor_sub(
                            XT[0:ka, 0 : w - 1], U[0:ka, 1:w], U[0:ka, 0 : w - 1]
                        )
                        nc.vector.memset(XT[0:ka, w - 1 : w], 0.0)

                    # ---- YT (PE): partition-dim forward diff -> PSUM ----
                    YT = psum.tile([128, w], f32, tag="YT")
                    my = wt[{(0, 0): "my", (1, 0): "myf",
                             (0, 1): "myl", (1, 1): "myfl"}[(first, last)]]
                    if do_yt:
                        for n0 in range(0, w, chunk):
                            nc.tensor.matmul(
                                YT[0:ka, n0 : n0 + chunk],
                                my[0:ku, 0:ka],
                                U[0:ku, n0 : n0 + chunk],
                            )

                    # ---- products (DVE) ----
                    # PA[q, s] = a[r0+q, s] * X[r0+q-1, s-1c]   s in [0, w+1)
                    PA = tmp.tile([128, w + 1], f32, tag="PA")
                    PB1 = tmp.tile([128, w + 1], f32, tag="PB1")
                    PB2 = tmp.tile([128, w], f32, tag="PB2")
                    PC = tmp.tile([128, w], f32, tag="PC")
                    if do_dve:
                        nc.vector.tensor_mul(
                            PA[0:ka, 1 : w + 1], A[0:ka, 1 : w + 1], XT[0:ka, 0:w]
                        )
                        nc.vector.tensor_mul(PA[0:ka, 0:1], A[0:ka, 0:1], XT[0:ka, 0:1])
                        # PB1[q, s] = b[r0+q, s] * Y[r0+q-1, s-1c]
                        nc.vector.tensor_mul(
                            PB1[0:ka, 1 : w + 1], Bt[0:ka, 1 : w + 1], YT[0:ka, 0:w]
                        )
                        nc.vector.tensor_mul(PB1[0:ka, 0:1], Bt[0:ka, 0:1], YT[0:ka, 0:1])
                        # PB2/PC stored at local col s-1, s in [1, w+1)
                        nc.vector.tensor_mul(
                            PB2[0:ka, 0:w], Bt[0:ka, 1 : w + 1], XT[0:ka, 0:w]
                        )
                        nc.vector.tensor_mul(PC[0:ka, 0:w], C[0:ka, 1 : w + 1], YT[0:ka, 0:w])

                    # ---- PSUM assembly (PE, fast-FP32 matmul) ----
                    OUTP = psum.tile([128, w], f32, tag="OUTP")
                    for n0 in (range(0, w, chunk) if do_pe else ()):
                        cw = min(chunk, w - n0)
                        o = OUTP[0:rt, n0 : n0 + cw]
                        mm = [
                            (wt["wu"][0:ka, 0:rt], U[0:ka, n0 : n0 + cw]),
                            (wt["wsp"][0:ka, 0:rt], PA[0:ka, n0 + 1 : n0 + 1 + cw]),
                            (wt["wsn"][0:ka, 0:rt], PA[0:ka, n0 : n0 + cw]),
                            (wt["wsp"][0:ka, 0:rt], PB1[0:ka, n0 + 1 : n0 + 1 + cw]),
                            (wt["wsn"][0:ka, 0:rt], PB1[0:ka, n0 : n0 + cw]),
                            (wt["wg"][0:ka, 0:rt], PB2[0:ka, n0 : n0 + cw]),
                            (wt["wg"][0:ka, 0:rt], PC[0:ka, n0 : n0 + cw]),
                        ]
                        for i, (lhsT, rhs) in enumerate(mm):
                            nc.tensor.matmul(
                                o,
                                lhsT,
                                rhs,
                                start=(i == 0),
                                stop=(i == len(mm) - 1),
                            )

                    # ---- PSUM -> SBUF (ACT), store ----
                    OS = tmp.tile([128, w], f32, tag="OS")
                    if do_act:
                        nc.scalar.copy(OS[0:rt, :], OUTP[0:rt, :])
                    else:
                        nc.vector.memset(OS[0:1, 0:4], 0.0)
                    if do_act and not do_pe:
                        nc.vector.memset(OUTP[0:1, 0:4], 0.0)
                    if do_dve and not do_yt:
                        nc.vector.memset(YT[0:1, 0:4], 0.0)
                    if do_pe and not do_dve:
                        for _t in (PA, PB1, PB2, PC):
                            nc.vector.memset(_t[0:1, 0:4], 0.0)
                    nc.sync.dma_start(out_d[ch, r0 : r0 + rt, :], OS[0:rt, :])

    nc.compile()
    return nc


def kernel(u, a, b, c, tau, grad_x, grad_y):
    from concourse.bass_utils import run_bass_kernel_spmd

    u = np.ascontiguousarray(np.asarray(u, dtype=np.float32))
    a = np.ascontiguousarray(np.asarray(a, dtype=np.float32))
    b = np.ascontiguousarray(np.asarray(b, dtype=np.float32))
    c = np.ascontiguousarray(np.asarray(c, dtype=np.float32))
    hx = float(np.asarray(grad_x)[0, 0, 1, 2])
    s = float(np.asarray(tau)) * hx * hx
    rt_last = H % R if H % R else R
    wts = _host_weights(s, rt_last)

    nc = _build_nc(N_CH, H, W, R, CHUNK)
    in_maps = [
        {"u": u[k], "a": a[k], "b": b[k], "c": c[k], "wts": wts}
        for k in range(N_CORES)
    ]
    res = run_bass_kernel_spmd(nc, in_maps, list(range(N_CORES)))
    return np.stack([res.results[k]["out"] for k in range(N_CORES)], axis=0)



# revision 2
# speedup vs baseline: 11.9660x; 11.9660x over previous
"""Trainium2 Bass kernel for nn_DiffusionBlock (anisotropic diffusion step).

Math (s = tau*hx^2; all index clamps from the replication padding):
  X[i,j] = u[i,j+1]-u[i,j] (0 at j=W-1),  Y[i,j] = u[i+1,j]-u[i,j] (0 at i=H-1)
  PF[i,j'] = a[i+1,j']*X[i,max(j'-1,0)] + b[i+1,j']*Y[i,max(j'-1,0)]  j' in [0,W+1)
  PG[r,m]  = b[r,m+1]*X[max(r-1,0),m]  + c[r,m+1]*Y[max(r-1,0),m]     r in [0,H], m in [0,W)
  out[i,j] = u[i,j] + s*(PF[i,j+1]-PF[i,j] + PG[i+1,j]-PG[i,j])

Distribution: pure batch data-parallel, one batch element per core (8 cores).

Per-core layout: everything bf16 (host pre-casts inputs once, like the
host-computed weights of a conv kernel; end-to-end rel err ~3e-3 vs the
2e-2 gate).  H=1024 rows split into G=2 groups of 512; within a group,
partition p owns RP=4 output rows rb=512g+4p..rb+3, flattened row-major in
the free dimension.  Row shifts are then free-dim offsets, so the whole
stencil runs on DVE/Pool with zero PE matmuls, and DMA moves multi-row
8-12KB descriptors instead of 4KB single rows (the descriptor count/size is
the DMA throughput limiter on this part: 4KB rows gave ~40 GB/s, this
layout with every transfer split across the SP and Act HWDGE queues
sustains the full pipeline).  Group halo rows come from one overlapping
strided DRAM read; the H-boundary clamp rows are materialized by two tiny
single-partition DMAs per channel.  Stores are lagged one iteration so they
never block the next iteration's prefetch.

Engines: Pool computes the X/Y diffs, DVE the products/diffs and the fused
final (t*s + u) via scalar_tensor_tensor; ACT and SP serve as the two DMA
queue issuers.  Compute fully hides under neither engine: steady state is
~26us per (channel, group) iteration, balanced between DVE and Pool.
"""

import numpy as np
import ml_dtypes

N_CORES = 8
N_CH = 2
H = 1024
W = 1024
RP = 4                 # output rows per partition per group
NP = 128               # partitions
GR = NP * RP           # rows per group (512)
G = H // GR            # groups per channel (2)
W2 = W + 2


def _build_nc(s: float, reps: int = 1, mode: str = "full"):
    import concourse.bacc as bacc
    import concourse.mybir as mybir
    import concourse.tile as tile
    from concourse.ap import AP

    bf16 = mybir.dt.bfloat16
    alu = mybir.AluOpType

    nc = bacc.Bacc()
    u_d = nc.dram_tensor("u", [N_CH, H, W], bf16, kind="ExternalInput")
    a_d = nc.dram_tensor("a", [N_CH, H + 2, W2], bf16, kind="ExternalInput")
    b_d = nc.dram_tensor("b", [N_CH, H + 2, W2], bf16, kind="ExternalInput")
    c_d = nc.dram_tensor("c", [N_CH, H + 2, W2], bf16, kind="ExternalInput")
    out_d = nc.dram_tensor("out", [N_CH, H, W], bf16, kind="ExternalOutput")

    do_compute = mode != "dma"

    with tile.TileContext(nc) as tc:
        with (
            tc.tile_pool(name="io", bufs=2) as io,
            tc.tile_pool(name="xy", bufs=2) as xy,
            tc.tile_pool(name="tmp", bufs=1) as tmp,
        ):
            pending_store = [None]

            def split_load(dst, src, p0=0):
                """dst[p0:p0+n] <- src ([n, f] DRAM AP), half per HWDGE queue."""
                pstride, n = src.ap[0][0], src.ap[0][1]
                flen = src.ap[1][1]
                hp = n // 2
                lo = AP(src.tensor, src.offset, [(pstride, hp), (1, flen)])
                hi = AP(src.tensor, src.offset + hp * pstride,
                        [(pstride, n - hp), (1, flen)])
                nc.sync.dma_start(dst[p0 : p0 + hp, :], lo)
                nc.scalar.dma_start(dst[p0 + hp : p0 + n, :], hi)

            def flush_store():
                if pending_store[0] is not None:
                    dst, src = pending_store[0]
                    hp = NP // 2
                    nc.sync.dma_start(
                        AP(dst.tensor, dst.offset,
                           [(dst.ap[0][0], hp), (1, dst.ap[1][1])]),
                        src[0:hp, :])
                    nc.scalar.dma_start(
                        AP(dst.tensor, dst.offset + hp * dst.ap[0][0],
                           [(dst.ap[0][0], NP - hp), (1, dst.ap[1][1])]),
                        src[hp:NP, :])
                    pending_store[0] = None

            for _rep in range(reps):
              for ch in range(N_CH):
                for g in range(G):
                    first = g == 0
                    last = g == G - 1
                    rb0 = GR * g          # group base row
                    # ---------------- tiles ----------------
                    U = io.tile([NP, (RP + 2) * W], bf16, tag="U")
                    Ab = io.tile([NP, RP * W2], bf16, tag="Ab")
                    Bb = io.tile([NP, (RP + 1) * W2], bf16, tag="Bb")
                    Cb = io.tile([NP, (RP + 1) * W2], bf16, tag="Cb")
                    OUT = io.tile([NP, RP * W], bf16, tag="OUT")

                    # ---------------- loads ----------------
                    # U slots t=-1..RP at offset (t+1)*W <- u rows rb-1..rb+RP
                    # (one overlapping strided AP; H-boundary rows handled by
                    # two tiny single-partition DMAs with a replicated row)
                    uch = u_d[ch]
                    if first:
                        split_load(
                            U,
                            AP(uch.tensor, uch.offset + (RP - 1) * W,
                               [(RP * W, NP - 1), (1, (RP + 2) * W)]),
                            p0=1)
                        nc.sync.dma_start(
                            U[0:1, W : (RP + 2) * W],
                            uch[0 : RP + 1, :].rearrange("r w -> (r w)").unsqueeze(0),
                        )
                        nc.sync.dma_start(U[0:1, 0:W], uch[0:1, :])
                    elif last:
                        split_load(
                            U,
                            AP(uch.tensor, uch.offset + (rb0 - 1) * W,
                               [(RP * W, NP - 1), (1, (RP + 2) * W)]),
                            p0=0)
                        nc.sync.dma_start(
                            U[NP - 1 : NP, 0 : (RP + 1) * W],
                            uch[H - RP - 1 : H, :].rearrange("r w -> (r w)").unsqueeze(0),
                        )
                        nc.sync.dma_start(
                            U[NP - 1 : NP, (RP + 1) * W : (RP + 2) * W],
                            uch[H - 1 : H, :],
                        )
                    else:
                        split_load(
                            U,
                            AP(uch.tensor, uch.offset + (rb0 - 1) * W,
                               [(RP * W, NP), (1, (RP + 2) * W)]))
                    # Ab slots t=1..RP at (t-1)*W2 <- a rows rb+1..rb+RP
                    ach = a_d[ch]
                    split_load(Ab,
                               AP(ach.tensor, ach.offset + (rb0 + 1) * W2,
                                  [(RP * W2, NP), (1, RP * W2)]))
                    # Bb/Cb slots r=0..RP at r*W2 <- b/c rows rb..rb+RP
                    bch = b_d[ch]
                    split_load(Bb,
                               AP(bch.tensor, bch.offset + rb0 * W2,
                                  [(RP * W2, NP), (1, (RP + 1) * W2)]))
                    cch = c_d[ch]
                    split_load(Cb,
                               AP(cch.tensor, cch.offset + rb0 * W2,
                                  [(RP * W2, NP), (1, (RP + 1) * W2)]))

                    # previous iteration's store goes after this iter's loads
                    flush_store()

                    out_ap = AP(out_d[ch].tensor, out_d[ch].offset + rb0 * W,
                                [(RP * W, NP), (1, RP * W)])
                    if not do_compute:
                        nc.vector.memset(OUT[0:1, 0:4], 0.0)
                        pending_store[0] = (out_ap, OUT[:, :])
                        continue

                    # ---------------- X, Y (Pool) ----------------
                    # X/Y slots t=-1..RP-1 at offset (t+1)*W
                    X = xy.tile([NP, (RP + 1) * W], bf16, tag="X")
                    Yt = xy.tile([NP, (RP + 1) * W], bf16, tag="Y")
                    x3 = X[:, :].rearrange("p (r w) -> p r w", w=W)
                    # flat diff; per-row col W-1 contamination is then
                    # overwritten by the strided memset
                    nc.gpsimd.tensor_tensor(
                        X[:, 0 : (RP + 1) * W - 1],
                        U[:, 1 : (RP + 1) * W],
                        U[:, 0 : (RP + 1) * W - 1],
                        alu.subtract,
                    )
                    nc.gpsimd.memset(x3[:, :, W - 1 : W], 0.0)
                    nc.gpsimd.tensor_tensor(
                        Yt[:, :],
                        U[:, W : (RP + 2) * W],
                        U[:, 0 : (RP + 1) * W],
                        alu.subtract,
                    )
                    if first:
                        # global row -1 clamp: Y[-1] := Y[0] on partition 0
                        nc.gpsimd.tensor_copy(Yt[0:1, 0:W], Yt[0:1, W : 2 * W])

                    # ---------------- PF, PG, out (DVE) ----------------
                    PF = tmp.tile([NP, RP * (W + 1)], bf16, tag="PF")
                    PG = tmp.tile([NP, (RP + 1) * W], bf16, tag="PG")
                    DF = tmp.tile([NP, RP * W], bf16, tag="DF")
                    DG = tmp.tile([NP, (RP + 1) * W], bf16, tag="DG")
                    OA = tmp.tile([NP, RP * W], bf16, tag="OA")

                    pf = PF[:, :].rearrange("p (r w) -> p r w", w=W + 1)
                    ab = Ab[:, :].rearrange("p (r w) -> p r w", w=W2)
                    bb = Bb[:, :].rearrange("p (r w) -> p r w", w=W2)
                    cb = Cb[:, :].rearrange("p (r w) -> p r w", w=W2)
                    xm = X[:, :].rearrange("p (r w) -> p r w", w=W)
                    ym = Yt[:, :].rearrange("p (r w) -> p r w", w=W)
                    df = DF[:, :].rearrange("p (r w) -> p r w", w=W)
                    dg = DG[:, :].rearrange("p (r w) -> p r w", w=W)
                    pg = PG[:, :].rearrange("p (r w) -> p r w", w=W)

                    # PF main (j'=1..W): a[t+1]*X[t] + b[t+1]*Y[t]
                    #   Ab row idx t-1 ; X slot t = xm row t+1
                    nc.vector.tensor_tensor(
                        pf[:, :, 1 : W + 1], ab[:, :, 1 : W + 1],
                        xm[:, 1 : RP + 1, :], alu.mult,
                    )
                    nc.vector.tensor_tensor(
                        df[:, :, :], bb[:, 1 : RP + 1, 1 : W + 1],
                        ym[:, 1 : RP + 1, :], alu.mult,
                    )
                    nc.vector.tensor_tensor(
                        pf[:, :, 1 : W + 1], pf[:, :, 1 : W + 1], df[:, :, :],
                        alu.add,
                    )
                    # PF edge col j'=0: a[t+1,0]*X[t,0] + b[t+1,0]*Y[t,0]
                    nc.vector.tensor_tensor(
                        pf[:, :, 0:1], ab[:, :, 0:1], xm[:, 1 : RP + 1, 0:1],
                        alu.mult,
                    )
                    nc.vector.tensor_tensor(
                        dg[:, 0:RP, 0:1], bb[:, 1 : RP + 1, 0:1],
                        ym[:, 1 : RP + 1, 0:1], alu.mult,
                    )
                    nc.vector.tensor_tensor(
                        pf[:, :, 0:1], pf[:, :, 0:1], dg[:, 0:RP, 0:1], alu.add,
                    )
                    # PG[r] = b[r]*Xc[r-1] + c[r]*Yc[r-1]; Xc[r-1] = xm row r
                    nc.vector.tensor_tensor(
                        pg[:, :, :], bb[:, :, 1 : W + 1], xm[:, :, :], alu.mult,
                    )
                    nc.vector.tensor_tensor(
                        dg[:, :, :], cb[:, :, 1 : W + 1], ym[:, :, :], alu.mult,
                    )
                    nc.vector.tensor_tensor(
                        pg[:, :, :], pg[:, :, :], dg[:, :, :], alu.add,
                    )
                    # DF = PF[:,1:] - PF[:,:W] ; DG = PG[1:] - PG[:RP]
                    nc.vector.tensor_tensor(
                        df[:, :, :], pf[:, :, 1 : W + 1], pf[:, :, 0:W],
                        alu.subtract,
                    )
                    nc.vector.tensor_tensor(
                        dg[:, 0:RP, :], pg[:, 1 : RP + 1, :], pg[:, 0:RP, :],
                        alu.subtract,
                    )
                    # OA = DF*s + U ; OUT = DG*s + OA
                    nc.vector.scalar_tensor_tensor(
                        OA[:, :], DF[:, :], float(s), U[:, W : (RP + 1) * W],
                        alu.mult, alu.add,
                    )
                    nc.vector.scalar_tensor_tensor(
                        OUT[:, :], DG[:, 0 : RP * W], float(s), OA[:, :],
                        alu.mult, alu.add,
                    )

                    pending_store[0] = (out_ap, OUT[:, :])
            flush_store()

    nc.compile()
    return nc


def _prep_inputs(u, a, b, c, tau, grad_x):
    """Host-side one-time prep: bf16 cast + scalar s = tau*hx^2."""
    bf = ml_dtypes.bfloat16
    ub = np.ascontiguousarray(np.asarray(u)).astype(bf)
    ab = np.ascontiguousarray(np.asarray(a)).astype(bf)
    bb = np.ascontiguousarray(np.asarray(b)).astype(bf)
    cb = np.ascontiguousarray(np.asarray(c)).astype(bf)
    hx = float(np.asarray(grad_x)[0, 0, 1, 2])
    s = float(np.asarray(tau)) * hx * hx
    return ub, ab, bb, cb, s


def kernel(u, a, b, c, tau, grad_x, grad_y):
    from concourse.bass_utils import run_bass_kernel_spmd

    ub, ab, bb, cb, s = _prep_inputs(u, a, b, c, tau, grad_x)
    nc = _build_nc(s)
    in_maps = [
        {"u": ub[k], "a": ab[k], "b": bb[k], "c": cb[k]}
        for k in range(N_CORES)
    ]
    res = run_bass_kernel_spmd(nc, in_maps, list(range(N_CORES)))
    return np.stack(
        [res.results[k]["out"].astype(np.float32) for k in range(N_CORES)], axis=0
    )
